# revision 22
# baseline (speedup 1.0000x reference)
"""Trainium2 Bass kernel for 3x3 same-padding Conv2d + bias (NCHW).

Problem: x[16,32,256,256] (*) weight[32,32,3,3] + bias[32] -> out[16,32,256,256]

Strategy (data-parallel over batch, 2 images per NeuronCore on 8 cores):
  - Host pre-shuffles x into the SBUF "slot" layout x_shuf[b][(g,ci)][s][258]:
    image row h lives in row-group g=(h+1)%4 at slot s=(h+1)//4; each slot is
    258 wide (zero pad col on each side) so the 3 horizontal conv taps are
    plain free-dim shifts.  Device input DMAs are fully contiguous.
  - Output computed in "quads" of 4 consecutive rows: PSUM tile [128, 256]
    with partitions = (r, co).  Quad u accumulates 6 matmuls (K=128, M=128,
    N=256): 3 horizontal taps kw for the slot-u window (rows 4u-1..4u+2) and
    3 for the slot-(u+1) window (rows 4u+3..4u+4; other weight rows zero).
  - Weight matrices (6 x [128,128], zero-padded per (g, r_out, kh) validity)
    are precomputed on the host from `weight`.
  - Matmuls run as float32r (full-rate fp32, reduced-precision multiplies).
  - PSUM -> SBUF staging copies alternate VectorE/ScalarE; the device writes
    out_shuf[b][(r,co)][q][w] (contiguous per partition) and the host
    unshuffles to NCHW and adds bias (exact for any bias).
"""
import sys

if "/opt/trn_rl_repo" not in sys.path:
    sys.path.insert(0, "/opt/trn_rl_repo")

import numpy as np

B, C, H, W = 16, 32, 256, 256
N_CORES = 8
PER = B // N_CORES          # batches per core
HW = H * W
NSLOT = H // 4 + 1          # 65 row slots
SLOTW = W + 2               # 258 padded columns per slot
NQ = H // 4                 # 64 quads per image
CHUNK = 8                   # quads per staging buffer / out DMA
# progressive input sub-tile sizes (quads) per batch: small first so the PE
# starts within a few us, large later for DMA efficiency
QSIZES = ([4, 8, 16, 36], [32, 32])

DT_KEY = "fp32r"            # "fp32r" (exact-ish) or "bf16" (half input DMA)

_cache = {}


def _get_nc(dt_key=None):
    dt_key = dt_key or DT_KEY
    if dt_key in _cache:
        return _cache[dt_key]
    import concourse.mybir as mybir
    import concourse.tile as tile
    import concourse.bass as bass
    from concourse import bacc

    DT = (mybir.dt.float32r if dt_key.startswith("fp32r")
          else mybir.dt.bfloat16)
    packed = dt_key.endswith("f")   # scheme F: 4 concurrent 64x64 PE tiles
    F32 = mybir.dt.float32

    nc = bacc.Bacc("TRN2", target_bir_lowering=False, debug=False,
                   num_devices=N_CORES)
    x_shuf = nc.dram_tensor("x_shuf", [PER, 128, NSLOT * SLOTW], DT,
                            kind="ExternalInput")
    w_taps = nc.dram_tensor("w_taps", [6, 128, 128], DT, kind="ExternalInput")
    out_shuf = nc.dram_tensor("out_shuf", [PER, 128, NQ * W], F32,
                              kind="ExternalOutput")

    assert all(sum(s) == NQ for s in QSIZES)
    n_xtiles = sum(len(s) for s in QSIZES)
    with tile.TileContext(nc) as tc:
        with (
            tc.tile_pool(name="xin", bufs=1) as xpool,  # unique tag per tile
            tc.tile_pool(name="wts", bufs=1) as wpool,
            tc.tile_pool(name="stage", bufs=3) as spool,
            tc.tile_pool(name="psum", bufs=4 if packed else 8,
                         space="PSUM") as ppool,
        ):
            w_t = wpool.tile([128, 6, 128], DT)
            nc.sync.dma_start(out=w_t[:],
                              in_=w_taps.ap().rearrange("t k m -> k t m"))

            # load both batches up front as contiguous sub-tiles with a
            # 1-slot overlap; progressive sizes so compute starts early
            xts = {}       # (b, tile_idx) -> (tile, start_quad)
            starts = {}
            for b in range(PER):
                q0 = 0
                for j, qsz in enumerate(QSIZES[b]):
                    xt = xpool.tile([128, qsz + 1, SLOTW], DT,
                                    tag=f"x_{b}_{j}")
                    lo = q0 * SLOTW
                    hi = lo + (qsz + 1) * SLOTW
                    nc.sync.dma_start(
                        out=xt[:],
                        in_=x_shuf.ap()[b, :, lo:hi]
                        .rearrange("p (s w) -> p s w", w=SLOTW))
                    xts[(b, j)] = (xt, q0)
                    q0 += qsz
                starts[b] = np.cumsum([0] + QSIZES[b]).tolist()

            # HAM warm-up: dummy matmuls during the first load so the PE
            # clock is already at 2.4 GHz when real work arrives
            wm = ppool.tile([128, W], F32, tag="ps1" if packed else "ps")
            for _ in range(24):
                nc.tensor.matmul(wm[0:64, 0:128], w_t[0:64, 0, 0:64],
                                 w_t[0:64, 1, :], start=True, stop=True)

            for b in range(PER):
                for k in range(NQ // CHUNK):
                    st = spool.tile([128, CHUNK, W], F32)
                    for ql in range(CHUNK):
                        u = k * CHUNK + ql
                        j = next(i for i in range(len(QSIZES[b]))
                                 if starts[b][i + 1] > u)
                        xt, q0 = xts[(b, j)]
                        lu = u - q0
                        if not packed:
                            ps = ppool.tile([128, W], F32)
                            for kw in range(3):
                                nc.tensor.matmul(ps[:], w_t[:, kw * 2, :],
                                                 xt[:, lu, kw:kw + W],
                                                 start=(kw == 0), stop=False)
                                nc.tensor.matmul(ps[:], w_t[:, kw * 2 + 1, :],
                                                 xt[:, lu + 1, kw:kw + W],
                                                 start=False, stop=(kw == 2))
                            nc.vector.tensor_copy(st[:, ql, :], ps[:])
                        else:
                            # scheme F: 4 concurrent 64x64 PE tiles per kw.
                            # ps1 <- A1 (rows 4u..4u+1 from K-groups 0-1) and
                            #        B2 (rows 4u+2..3 from slot u+1 groups 0-1)
                            # ps2 <- A2 (rows 4u..4u+1 from K-groups 2-3) and
                            #        B1 (rows 4u+2..3 from slot u groups 2-3)
                            # each PSUM bank is written by one PE row-strip;
                            # two quads share a bank (512-wide halves) so the
                            # ACT copy + DVE add run once per quad pair.
                            if ql % 2 == 0:
                                ps1 = ppool.tile([128, 2, W], F32)
                                ps2 = ppool.tile([128, 2, W], F32)
                            e = ql % 2
                            for kw in range(3):
                                s0, s1 = (kw == 0), (kw == 2)
                                nc.tensor.matmul(ps1[0:64, e, :],
                                                 w_t[0:64, kw * 2, 0:64],
                                                 xt[0:64, lu, kw:kw + W],
                                                 start=s0, stop=s1)
                                nc.tensor.matmul(ps2[0:64, e, :],
                                                 w_t[64:128, kw * 2, 0:64],
                                                 xt[64:128, lu, kw:kw + W],
                                                 start=s0, stop=s1)
                                nc.tensor.matmul(ps2[64:128, e, :],
                                                 w_t[64:128, kw * 2, 64:128],
                                                 xt[64:128, lu, kw:kw + W],
                                                 start=s0, stop=s1)
                                nc.tensor.matmul(ps1[64:128, e, :],
                                                 w_t[0:64, kw * 2 + 1, 64:128],
                                                 xt[0:64, lu + 1, kw:kw + W],
                                                 start=s0, stop=s1)
                            if ql % 2 == 1:
                                tmp = spool.tile([128, 2, W], F32,
                                                 tag="ps2tmp")
                                nc.scalar.copy(tmp[:], ps2[:])
                                nc.vector.tensor_add(st[:, ql - 1:ql + 1, :],
                                                     ps1[:], tmp[:])
                    # contiguous per-partition store of CHUNK quads; issued
                    # on the ACT HWDGE ring so stores never queue ahead of
                    # input loads (which use the SP ring)
                    dst = bass.AP(out_shuf, b * 128 * NQ * W + k * CHUNK * W,
                                  [[NQ * W, 128], [1, CHUNK * W]])
                    # packed mode keeps ACT busy with PSUM copies, so issue
                    # stores from the idle GpSimd (SWDGE) there instead
                    store_eng = nc.gpsimd if packed else nc.scalar
                    store_eng.dma_start(
                        out=dst,
                        in_=st[:].rearrange("p q w -> p (q w)"))

    nc.compile()
    _cache[dt_key] = nc
    return nc


def _make_w_taps(weight):
    """Zero-padded stationary matrices w_taps[kw*2+part][(g,ci), (r,co)]."""
    w_taps = np.zeros((6, 128, 128), dtype=np.float32)
    for kw in range(3):
        for g in range(4):
            for r in range(4):
                kh0 = g - r              # window W_u (input row 4u+g-1)
                if 0 <= kh0 <= 2:
                    w_taps[kw * 2, g * 32:(g + 1) * 32, r * 32:(r + 1) * 32] = \
                        weight[:, :, kh0, kw].T
                kh1 = g - r + 4          # window W_{u+1} (input row 4u+g+3)
                if 0 <= kh1 <= 2:
                    w_taps[kw * 2 + 1, g * 32:(g + 1) * 32, r * 32:(r + 1) * 32] = \
                        weight[:, :, kh1, kw].T
    return w_taps


def _shuffle_x(x, np_dt=np.float32):
    """x[B,C,H,W] -> x_shuf[B,128,NSLOT,SLOTW]: row h -> (group (h+1)%4,
    slot (h+1)//4), cols 1..W, zero pads elsewhere."""
    xs = np.zeros((B, 128, NSLOT, SLOTW), dtype=np_dt)
    # group g, slot s holds row 4s+g-1
    xs[:, 0:32, 1:NSLOT, 1:W + 1] = x[:, :, 3::4, :].astype(np_dt)
    xs[:, 32:64, 0:NSLOT - 1, 1:W + 1] = x[:, :, 0::4, :].astype(np_dt)
    xs[:, 64:96, 0:NSLOT - 1, 1:W + 1] = x[:, :, 1::4, :].astype(np_dt)
    xs[:, 96:128, 0:NSLOT - 1, 1:W + 1] = x[:, :, 2::4, :].astype(np_dt)
    return xs.reshape(B, 128, NSLOT * SLOTW)


def _unshuffle_out(chunks):
    """chunks: list of PER-core arrays [PER,128,NQ*W] -> out[B,C,H,W]."""
    o = np.concatenate(chunks, axis=0)              # [B, 128, NQ*W]
    o = o.reshape(B, 4, C, NQ, W)                   # [(r c), q, w]
    o = o.transpose(0, 2, 3, 1, 4)                  # [B, C, q, r, w]
    return np.ascontiguousarray(o.reshape(B, C, H, W))


def _np_dt(dt_key):
    if dt_key == "fp32r":
        return np.float32
    import ml_dtypes
    return ml_dtypes.bfloat16


def make_in_maps(x, weight, dt_key=None):
    dt_key = dt_key or DT_KEY
    np_dt = _np_dt(dt_key)
    w_taps = _make_w_taps(np.asarray(weight, dtype=np.float32)).astype(np_dt)
    x_shuf = _shuffle_x(np.asarray(x, dtype=np.float32), np_dt)
    return [{"x_shuf": x_shuf[c * PER:(c + 1) * PER], "w_taps": w_taps}
            for c in range(N_CORES)]


def kernel(x, weight, bias):
    from concourse.bass_utils import run_bass_kernel_spmd

    bias = np.asarray(bias, dtype=np.float32)
    nc = _get_nc()
    in_maps = make_in_maps(x, weight)
    res = run_bass_kernel_spmd(nc, in_maps, list(range(N_CORES)))
    out = _unshuffle_out([res.results[c]["out_shuf"] for c in range(N_CORES)])
    out += bias.reshape(1, C, 1, 1)
    return out


# revision 33
# speedup vs baseline: 1.6325x; 1.6325x over previous
"""Trainium2 Bass kernel for 3x3 same-padding Conv2d + bias (NCHW).

Problem: x[16,32,256,256] (*) weight[32,32,3,3] + bias[32] -> out[16,32,256,256]

Strategy (data-parallel over batch, 2 images per NeuronCore on 8 cores):
  - Host pre-shuffles x into an SBUF "slot" layout x_shuf[(g,ci)][s][b2][258]:
    image row h lives in row-group g=(h+1)%4 at slot s=(h+1)//4; the core's
    two batches are interleaved per slot; each row is 258 wide (zero pad col
    on each side) so the 3 horizontal conv taps are plain free-dim shifts.
    Device input DMAs are fully contiguous.
  - Output computed in "quads" of 4 consecutive rows for both batches at
    once: PSUM tile [128, 2, 256] (a full 2 KB bank) with partitions =
    (r, co).  Quad u accumulates 6 matmuls (K=128, M=128, N=512 via a 2-dim
    moving AP): 3 horizontal taps kw for the slot-u window (rows 4u-1..4u+2)
    and 3 for the slot-(u+1) window (rows 4u+3..4u+4; other weight rows 0).
  - Weight matrices (6 x [128,128], zero-padded per (g, r_out, kh) validity)
    are precomputed on the host from `weight`.
  - Matmuls run as float32r (full-rate fp32, reduced-precision multiplies)
    by default; bf16 variants halve the input traffic at ~2e-3 rel error.
  - PSUM -> SBUF staging copies alternate VectorE/ScalarE; the device writes
    out_shuf[(r,co)][q][b2][w] (contiguous per partition) and the host
    unshuffles to NCHW and adds bias (exact for any bias).
"""
import sys

if "/opt/trn_rl_repo" not in sys.path:
    sys.path.insert(0, "/opt/trn_rl_repo")

import numpy as np

B, C, H, W = 16, 32, 256, 256
N_CORES = 8
PER = B // N_CORES          # batches per core (interleaved in the free dim)
HW = H * W
NSLOT = H // 4 + 1          # 65 row slots
SLOTW = W + 2               # 258 padded columns per slot
SLOTF = PER * SLOTW         # free-dim elements per slot (both batches)
NQ = H // 4                 # 64 quads
CHUNK = 4                   # quads per staging buffer / out DMA (1 MB)
# progressive input sub-tile sizes (quads): small first so the PE starts
# within a few us, large later for DMA efficiency
QSIZES = [4, 8, 16, 18, 18]

DT_KEY = "fp32r"    # "fp32r" | "bf16" | "bf16f" (packed 64x64 PE tiles)

_cache = {}


def _get_nc(dt_key=None):
    dt_key = dt_key or DT_KEY
    if dt_key in _cache:
        return _cache[dt_key]
    import concourse.mybir as mybir
    import concourse.tile as tile
    import concourse.bass as bass
    from concourse import bacc

    DT = (mybir.dt.float32r if dt_key.startswith("fp32r")
          else mybir.dt.bfloat16)
    packed = dt_key == "bf16f"      # scheme F: 4 concurrent 64x64 PE tiles
    F32 = mybir.dt.float32

    nc = bacc.Bacc("TRN2", target_bir_lowering=False, debug=False,
                   num_devices=N_CORES)
    x_shuf = nc.dram_tensor("x_shuf", [128, NSLOT * SLOTF], DT,
                            kind="ExternalInput")
    w_taps = nc.dram_tensor("w_taps", [6, 128, 128], DT, kind="ExternalInput")
    out_shuf = nc.dram_tensor("out_shuf", [128, NQ * PER * W], F32,
                              kind="ExternalOutput")

    assert sum(QSIZES) == NQ
    with tile.TileContext(nc) as tc:
        with (
            tc.tile_pool(name="xin", bufs=1) as xpool,  # unique tag per tile
            tc.tile_pool(name="wts", bufs=1) as wpool,
            tc.tile_pool(name="stage", bufs=3) as spool,
            tc.tile_pool(name="psum", bufs=4 if packed else 8,
                         space="PSUM") as ppool,
        ):
            # input sub-tiles with a 1-slot overlap, loaded as fully
            # contiguous DMAs; the weight load is issued right after the
            # first x sub-tile so it doesn't delay the PE's first real work
            w_t = wpool.tile([128, 6, 128], DT)
            xts = []       # (tile, start_quad)
            q0 = 0
            for j, qsz in enumerate(QSIZES):
                xt = xpool.tile([128, qsz + 1, PER, SLOTW], DT, tag=f"x{j}")
                lo = q0 * SLOTF
                hi = lo + (qsz + 1) * SLOTF
                nc.sync.dma_start(
                    out=xt[:],
                    in_=x_shuf.ap()[:, lo:hi]
                    .rearrange("p (s b w) -> p s b w", b=PER, w=SLOTW))
                xts.append((xt, q0))
                q0 += qsz
                if j == 0:
                    nc.sync.dma_start(
                        out=w_t[:],
                        in_=w_taps.ap().rearrange("t k m -> k t m"))
            starts = np.cumsum([0] + QSIZES).tolist()

            # HAM warm-up: dummy matmuls during the first load so the PE
            # clock is already at 2.4 GHz when real work arrives (N=256 so
            # float32r runs at full rate too)
            wm = ppool.tile([128, PER, W], F32, tag="ps1" if packed else "ps")
            for _ in range(24):
                nc.tensor.matmul(wm[0:64, 0, :], w_t[0:64, 0, 0:64],
                                 w_t[0:64, 0:2, :], start=True, stop=True)

            for k in range(NQ // CHUNK):
                st = spool.tile([128, CHUNK, PER, W], F32)
                for ql in range(CHUNK):
                    u = k * CHUNK + ql
                    j = next(i for i in range(len(QSIZES))
                             if starts[i + 1] > u)
                    xt, tq0 = xts[j]
                    lu = u - tq0
                    if not packed:
                        ps = ppool.tile([128, PER, W], F32)
                        for kw in range(3):
                            nc.tensor.matmul(ps[:], w_t[:, kw * 2, :],
                                             xt[:, lu, :, kw:kw + W],
                                             start=(kw == 0), stop=False)
                            nc.tensor.matmul(ps[:], w_t[:, kw * 2 + 1, :],
                                             xt[:, lu + 1, :, kw:kw + W],
                                             start=False, stop=(kw == 2))
                        if ql % 2 == 0:
                            nc.vector.tensor_copy(st[:, ql, :, :], ps[:])
                        else:
                            nc.scalar.copy(st[:, ql, :, :], ps[:])
                    else:
                        # scheme F: 4 concurrent 64x64 PE tiles per kw.
                        # ps1 <- A1 (rows 4u..4u+1 from K-groups 0-1) and
                        #        B2 (rows 4u+2..3 from slot u+1 groups 0-1)
                        # ps2 <- A2 (rows 4u..4u+1 from K-groups 2-3) and
                        #        B1 (rows 4u+2..3 from slot u groups 2-3)
                        # each PSUM bank is written by one PE row-strip.
                        ps1 = ppool.tile([128, PER, W], F32)
                        ps2 = ppool.tile([128, PER, W], F32)
                        for kw in range(3):
                            s0, s1 = (kw == 0), (kw == 2)
                            nc.tensor.matmul(ps1[0:64, :, :],
                                             w_t[0:64, kw * 2, 0:64],
                                             xt[0:64, lu, :, kw:kw + W],
                                             start=s0, stop=s1)
                            nc.tensor.matmul(ps2[0:64, :, :],
                                             w_t[64:128, kw * 2, 0:64],
                                             xt[64:128, lu, :, kw:kw + W],
                                             start=s0, stop=s1)
                            nc.tensor.matmul(ps2[64:128, :, :],
                                             w_t[64:128, kw * 2, 64:128],
                                             xt[64:128, lu, :, kw:kw + W],
                                             start=s0, stop=s1)
                            nc.tensor.matmul(ps1[64:128, :, :],
                                             w_t[0:64, kw * 2 + 1, 64:128],
                                             xt[0:64, lu + 1, :, kw:kw + W],
                                             start=s0, stop=s1)
                        tmp = spool.tile([128, PER, W], F32, tag="ps2tmp")
                        nc.scalar.copy(tmp[:], ps2[:])
                        nc.vector.tensor_add(st[:, ql, :, :], ps1[:], tmp[:])
                # contiguous per-partition store of CHUNK quads, issued on
                # the GpSimd (SWDGE) ring so stores never queue ahead of
                # input loads (SP ring) or PSUM copies (ACT)
                dst = bass.AP(out_shuf, k * CHUNK * PER * W,
                              [[NQ * PER * W, 128], [1, CHUNK * PER * W]])
                nc.gpsimd.dma_start(
                    out=dst,
                    in_=st[:].rearrange("p q b w -> p (q b w)"))

    nc.compile()
    _cache[dt_key] = nc
    return nc


def _make_w_taps(weight):
    """Zero-padded stationary matrices w_taps[kw*2+part][(g,ci), (r,co)]."""
    w_taps = np.zeros((6, 128, 128), dtype=np.float32)
    for kw in range(3):
        for g in range(4):
            for r in range(4):
                kh0 = g - r              # window W_u (input row 4u+g-1)
                if 0 <= kh0 <= 2:
                    w_taps[kw * 2, g * 32:(g + 1) * 32, r * 32:(r + 1) * 32] = \
                        weight[:, :, kh0, kw].T
                kh1 = g - r + 4          # window W_{u+1} (input row 4u+g+3)
                if 0 <= kh1 <= 2:
                    w_taps[kw * 2 + 1, g * 32:(g + 1) * 32, r * 32:(r + 1) * 32] = \
                        weight[:, :, kh1, kw].T
    return w_taps


def _np_dt(dt_key):
    if dt_key.startswith("fp32r"):
        return np.float32
    import ml_dtypes
    return ml_dtypes.bfloat16


def _shuffle_x(x, np_dt=np.float32):
    """x[B,C,H,W] -> per-core x_shuf[N_CORES,128,NSLOT,PER,SLOTW]: row h ->
    (group (h+1)%4, slot (h+1)//4), batch pair interleaved, cols 1..W data,
    zero pads elsewhere."""
    xc = x.reshape(N_CORES, PER, C, H, W)
    xs = np.zeros((N_CORES, 128, NSLOT, PER, SLOTW), dtype=np_dt)
    # group g, slot s holds row 4s+g-1
    xs[:, 0:32, 1:NSLOT, :, 1:W + 1] = \
        xc[:, :, :, 3::4, :].transpose(0, 2, 3, 1, 4).astype(np_dt)
    xs[:, 32:64, 0:NSLOT - 1, :, 1:W + 1] = \
        xc[:, :, :, 0::4, :].transpose(0, 2, 3, 1, 4).astype(np_dt)
    xs[:, 64:96, 0:NSLOT - 1, :, 1:W + 1] = \
        xc[:, :, :, 1::4, :].transpose(0, 2, 3, 1, 4).astype(np_dt)
    xs[:, 96:128, 0:NSLOT - 1, :, 1:W + 1] = \
        xc[:, :, :, 2::4, :].transpose(0, 2, 3, 1, 4).astype(np_dt)
    return xs.reshape(N_CORES, 128, NSLOT * SLOTF)


def _unshuffle_out(chunks):
    """chunks: per-core [128, NQ*PER*W] -> out[B,C,H,W]."""
    o = np.stack(chunks, axis=0)                  # [8, 128, NQ*PER*W]
    o = o.reshape(N_CORES, 4, C, NQ, PER, W)      # [c, r, co, q, b2, w]
    o = o.transpose(0, 4, 2, 3, 1, 5)             # [c, b2, co, q, r, w]
    return np.ascontiguousarray(o.reshape(B, C, H, W))


def make_in_maps(x, weight, dt_key=None):
    dt_key = dt_key or DT_KEY
    np_dt = _np_dt(dt_key)
    w_taps = _make_w_taps(np.asarray(weight, dtype=np.float32)).astype(np_dt)
    x_shuf = _shuffle_x(np.asarray(x, dtype=np.float32), np_dt)
    return [{"x_shuf": x_shuf[c], "w_taps": w_taps} for c in range(N_CORES)]


def kernel(x, weight, bias):
    from concourse.bass_utils import run_bass_kernel_spmd

    bias = np.asarray(bias, dtype=np.float32)
    nc = _get_nc()
    in_maps = make_in_maps(x, weight)
    res = run_bass_kernel_spmd(nc, in_maps, list(range(N_CORES)))
    out = _unshuffle_out([res.results[c]["out_shuf"] for c in range(N_CORES)])
    out += bias.reshape(1, C, 1, 1)
    return out
